# revision 7
# baseline (speedup 1.0000x reference)
"""BatchGRU (bidirectional, uniform 48-length segments) Trainium2 Bass kernel.

Problem: nn_BatchGRU_4844723110205.
  x [98304, 300] f32, batch = repeat(arange(2048), 48), bidirectional GRU
  (H=300) over the 48-step sequences with h0 = per-molecule max of raw x,
  message = relu(x + bias).  Output y [98304, 600] f32 (fwd || bwd).

Sharding: data-parallel over molecules, 256 molecules per core x 8 cores.

Per-core device algorithm (transposed "h-on-partitions" layout):
  - prologue: DMA x rows, PE-transpose into xT chunks [100, 12288] (f32r),
    h0 via free-axis max-reduce, relu(x+bias) in place.
  - recurrence (48 steps x 2 directions, interleaved): per step-dir
    54 f32r matmuls (input projection + hidden matmul accumulated in PSUM),
    gate math with per-chunk biases split across ACT/DVE/GPSIMD, in-place
    f32r state update, PE transposes of h_new to row layout, DMA y out.
"""
import numpy as np
from contextlib import ExitStack

try:
    import jax as _jax
    _jax.config.update("jax_compilation_cache_dir", "/root/problem/jax_cache")
    _jax.config.update("jax_persistent_cache_min_compile_time_secs", 10.0)
    _jax.config.update("jax_persistent_cache_min_entry_size_bytes", 0)
except Exception:
    pass

import concourse.bacc as bacc
import concourse.tile as tile
from concourse import mybir
from concourse.bass_utils import run_bass_kernel_spmd

f32 = mybir.dt.float32
f32r = mybir.dt.float32r
AF = mybir.ActivationFunctionType
ALU = mybir.AluOpType

H = 300        # hidden size
HC = 100       # h-dim chunk
NK = 3         # chunks of H
G = 3 * H      # gates (900)
L = 48         # sequence length
B = 2048       # molecules total
NCORES = 8
BLOC = B // NCORES          # 256 molecules per core
NLOC = BLOC * L             # 12288 rows per core
RT = NLOC // 128            # 96 row-tiles in prologue

_cached = {}


def build_program():
    if "nc" in _cached:
        return _cached["nc"]
    nc = bacc.Bacc("TRN2", target_bir_lowering=False, debug=False,
                   dynamic_dma_scratch_size=512)

    x_d = nc.declare_dram_parameter("x", [NLOC, H], f32, isOutput=False)
    wx_f_d = nc.declare_dram_parameter("wx_f", [NK, HC, G], f32, isOutput=False)
    wh_f_d = nc.declare_dram_parameter("wh_f", [NK, HC, G], f32, isOutput=False)
    wx_b_d = nc.declare_dram_parameter("wx_b", [NK, HC, G], f32, isOutput=False)
    wh_b_d = nc.declare_dram_parameter("wh_b", [NK, HC, G], f32, isOutput=False)
    # gbias[:, dir, j]: j 0-5 = rz bias (b_ih+b_hh), 6-8 = xn bias (b_ih_n),
    # 9-11 = hn bias (b_hh_n)
    gb_d = nc.declare_dram_parameter("gbias", [HC, 2, 12], f32, isOutput=False)
    brelu_d = nc.declare_dram_parameter("brelu", [HC, NK], f32, isOutput=False)
    y_d = nc.declare_dram_parameter("y", [NLOC, 2 * H], f32, isOutput=True)

    y_r = y_d[:].rearrange("(m l) c -> m l c", l=L)  # [256, 48, 600]

    with tile.TileContext(nc) as tc:
        with ExitStack() as ctx:
            consts = ctx.enter_context(tc.tile_pool(name="consts", bufs=1))

            # ---- resident tensors ----
            xT = [consts.tile([HC, NLOC], f32r, name=f"xT{k}") for k in range(NK)]
            wx_r = {d: consts.tile([HC, NK, G], f32r, name=f"wx_r_{d}") for d in "fb"}
            wh_r = {d: consts.tile([HC, NK, G], f32r, name=f"wh_r_{d}") for d in "fb"}
            gb_sb = consts.tile([HC, 2, 12], f32)
            brelu_sb = consts.tile([HC, NK], f32)
            id_f32 = consts.tile([128, 128], f32)
            id_f32r = consts.tile([HC, HC], f32r)
            state = {d: consts.tile([HC, NK, BLOC], f32r, name=f"state_{d}") for d in "fb"}

            # ---- identities ----
            with tc.tile_pool(name="idp", bufs=1) as idp:
                rowi = idp.tile([128, 1], mybir.dt.int32)
                coli = idp.tile([128, 128], mybir.dt.int32)
                nc.gpsimd.iota(rowi, pattern=[[0, 1]], base=0, channel_multiplier=1)
                nc.gpsimd.iota(coli, pattern=[[1, 128]], base=0, channel_multiplier=0)
                rowf = idp.tile([128, 1], f32)
                colf = idp.tile([128, 128], f32)
                nc.vector.tensor_copy(out=rowf, in_=rowi)
                nc.vector.tensor_copy(out=colf, in_=coli)
                nc.vector.tensor_scalar(out=id_f32, in0=colf, scalar1=rowf,
                                        scalar2=None, op0=ALU.is_equal)
                nc.scalar.activation(out=id_f32r, in_=id_f32[:HC, :HC], func=AF.Copy)

            nc.sync.dma_start(out=gb_sb, in_=gb_d[:])
            nc.sync.dma_start(out=brelu_sb, in_=brelu_d[:])

            # ---- weights: DMA f32 staging -> ACT rounding cast to f32r ----
            with tc.tile_pool(name="wstage", bufs=2) as wstage:
                for d in "fb":
                    wx_dram = wx_f_d if d == "f" else wx_b_d
                    wh_dram = wh_f_d if d == "f" else wh_b_d
                    st_x = wstage.tile([HC, NK, G], f32, tag="ws", name=f"stx_{d}")
                    nc.sync.dma_start(out=st_x, in_=wx_dram[:].rearrange("k p g -> p k g"))
                    nc.scalar.activation(out=wx_r[d], in_=st_x, func=AF.Copy)
                    st_h = wstage.tile([HC, NK, G], f32, tag="ws", name=f"sth_{d}")
                    nc.sync.dma_start(out=st_h, in_=wh_dram[:].rearrange("k p g -> p k g"))
                    nc.scalar.activation(out=wh_r[d], in_=st_h, func=AF.Copy)

            # ---- prologue: x -> xT (transposed, f32r), h0, relu ----
            with tc.tile_pool(name="xstage", bufs=4) as xstage, \
                 tc.tile_pool(name="tps", bufs=4, space="PSUM") as tps:
                for rt in range(RT):
                    x_nat = xstage.tile([128, H], f32, tag="xn")
                    nc.sync.dma_start(out=x_nat, in_=x_d[rt * 128:(rt + 1) * 128, :])
                    for k in range(NK):
                        ps = tps.tile([HC, 128], f32, tag="tp")
                        nc.tensor.transpose(out=ps, in_=x_nat[:, k * HC:(k + 1) * HC],
                                            identity=id_f32)
                        nc.scalar.activation(out=xT[k][:, rt * 128:(rt + 1) * 128],
                                             in_=ps, func=AF.Copy)

            # h0 (raw x, pre-relu) straight into fwd state; copy to bwd state
            for k in range(NK):
                nc.vector.tensor_reduce(
                    out=state["f"][:, k, :],
                    in_=xT[k].bitcast(f32).rearrange("p (m l) -> p m l", l=L),
                    axis=mybir.AxisListType.X, op=ALU.max)
            nc.vector.tensor_copy(out=state["b"], in_=state["f"].bitcast(f32))

            # relu(x + bias) in place on xT (ACT rounds to f32r)
            for k in range(NK):
                nc.scalar.activation(out=xT[k], in_=xT[k].bitcast(f32), func=AF.Relu,
                                     bias=brelu_sb[:, k:k + 1], scale=1.0)

            xT_ml = [xT[k].rearrange("p (m l) -> p m l", l=L) for k in range(NK)]

            # ---- recurrence pools ----
            rz_pool = ctx.enter_context(tc.tile_pool(name="rzp", bufs=1, space="PSUM"))
            nn_pool = ctx.enter_context(tc.tile_pool(name="nnp", bufs=1, space="PSUM"))
            yp_pool = ctx.enter_context(tc.tile_pool(name="ypp", bufs=1, space="PSUM"))
            gates = ctx.enter_context(tc.tile_pool(name="gates", bufs=1))
            youts = ctx.enter_context(tc.tile_pool(name="youts", bufs=2))

            for s in range(L):
                for d in "fb":
                    t = s if d == "f" else L - 1 - s
                    wx, wh, st = wx_r[d], wh_r[d], state[d]
                    dcol = 0 if d == "f" else 1

                    rz_ps = rz_pool.tile([HC, 6, BLOC], f32, tag="rz", name=f"rz_{d}{s}")
                    nn_ps = nn_pool.tile([HC, 6, BLOC], f32, tag="nn", name=f"nn_{d}{s}")
                    # r,z gates: 6 M-chunks, x-side + h-side accumulated
                    for j in range(6):
                        gsl = slice(j * HC, (j + 1) * HC)
                        for k in range(NK):
                            nc.tensor.matmul(out=rz_ps[:, j, :],
                                             lhsT=wx[:, k, gsl],
                                             rhs=xT_ml[k][:, :, t],
                                             start=(k == 0), stop=False)
                        for k in range(NK):
                            nc.tensor.matmul(out=rz_ps[:, j, :],
                                             lhsT=wh[:, k, gsl],
                                             rhs=st[:, k, :],
                                             start=False, stop=(k == NK - 1))
                    # n gate: xn into nn_ps[:,0:3], hn into nn_ps[:,3:6]
                    for j in range(NK):
                        gsl = slice(600 + j * HC, 700 + j * HC)
                        for k in range(NK):
                            nc.tensor.matmul(out=nn_ps[:, j, :],
                                             lhsT=wx[:, k, gsl],
                                             rhs=xT_ml[k][:, :, t],
                                             start=(k == 0), stop=(k == NK - 1))
                        for k in range(NK):
                            nc.tensor.matmul(out=nn_ps[:, 3 + j, :],
                                             lhsT=wh[:, k, gsl],
                                             rhs=st[:, k, :],
                                             start=(k == 0), stop=(k == NK - 1))

                    # gate math
                    r_s = gates.tile([HC, NK, BLOC], f32, tag="rs", name=f"rs_{d}{s}")
                    t1 = gates.tile([HC, NK, BLOC], f32, tag="t1", name=f"t1_{d}{s}")
                    n_s = gates.tile([HC, NK, BLOC], f32, tag="ns", name=f"ns_{d}{s}")
                    # r_j = sigmoid(rz_j + brz_j) -> sbuf; z: in place in psum
                    for j in range(NK):
                        nc.scalar.activation(out=r_s[:, j, :], in_=rz_ps[:, j, :],
                                             func=AF.Sigmoid,
                                             bias=gb_sb[:, dcol, j:j + 1])
                    for j in range(NK):
                        nc.scalar.activation(out=rz_ps[:, 3 + j, :],
                                             in_=rz_ps[:, 3 + j, :], func=AF.Sigmoid,
                                             bias=gb_sb[:, dcol, 3 + j:4 + j])
                    # t1_j = (hn_j + bhn_j) * r_j
                    for j in range(NK):
                        nc.vector.scalar_tensor_tensor(
                            out=t1[:, j, :], in0=nn_ps[:, 3 + j, :],
                            scalar=gb_sb[:, dcol, 9 + j:10 + j],
                            in1=r_s[:, j, :], op0=ALU.add, op1=ALU.mult)
                    # t1_j = (t1_j + bxn_j) + xn_j
                    for j in range(NK):
                        nc.vector.scalar_tensor_tensor(
                            out=t1[:, j, :], in0=t1[:, j, :],
                            scalar=gb_sb[:, dcol, 6 + j:7 + j],
                            in1=nn_ps[:, j, :], op0=ALU.add, op1=ALU.add)
                    # n = tanh(t1)
                    nc.scalar.activation(out=n_s, in_=t1, func=AF.Tanh)
                    # t1 = h_old - n   (gpsimd, sbuf only)
                    nc.gpsimd.tensor_sub(out=t1, in0=st.bitcast(f32), in1=n_s)
                    # t1 = z * t1      (z from psum)
                    nc.vector.tensor_mul(out=t1, in0=rz_ps[:, 3:6, :], in1=t1)
                    # h_new = n + t1 -> state (DVE write rounds to f32r)
                    nc.vector.tensor_add(out=st, in0=n_s, in1=t1)

                    # y output: transpose h_new [100,3,256] -> rows [256, 300]
                    for half in range(2):
                        y_ps = yp_pool.tile([128, NK, HC], f32r, tag=f"yp{half}",
                                            name=f"yp{half}_{d}{s}")
                        for j in range(NK):
                            nc.tensor.transpose(
                                out=y_ps[:, j, :],
                                in_=st[:, j, half * 128:(half + 1) * 128],
                                identity=id_f32r)
                        y_sb = youts.tile([128, H], f32, tag=f"ys{half}",
                                          name=f"ys{half}_{d}{s}")
                        nc.scalar.activation(
                            out=y_sb,
                            in_=y_ps.bitcast(f32).rearrange("p a b -> p (a b)"),
                            func=AF.Copy)
                        nc.sync.dma_start(
                            out=y_r[half * 128:(half + 1) * 128, t,
                                    dcol * H:(dcol + 1) * H],
                            in_=y_sb)

    nc.compile()
    _cached["nc"] = nc
    return nc


def _prep_shared_inputs(bias, w_ih_f, w_hh_f, b_ih_f, b_hh_f,
                        w_ih_b, w_hh_b, b_ih_b, b_hh_b):
    def pack_w(w):
        # [k, p, g] = w[g, k*100+p]
        return np.ascontiguousarray(
            np.asarray(w, np.float32).T.reshape(NK, HC, G))

    def gbias_dir(b_ih, b_hh):
        cols = np.empty((HC, 12), np.float32)
        cols[:, 0:6] = (b_ih[:600] + b_hh[:600]).reshape(6, HC).T
        cols[:, 6:9] = b_ih[600:].reshape(NK, HC).T
        cols[:, 9:12] = b_hh[600:].reshape(NK, HC).T
        return cols

    gb = np.stack([gbias_dir(np.asarray(b_ih_f, np.float64),
                             np.asarray(b_hh_f, np.float64)).astype(np.float32),
                   gbias_dir(np.asarray(b_ih_b, np.float64),
                             np.asarray(b_hh_b, np.float64)).astype(np.float32)],
                  axis=1)  # [100, 2, 12]
    return {
        "wx_f": pack_w(w_ih_f), "wh_f": pack_w(w_hh_f),
        "wx_b": pack_w(w_ih_b), "wh_b": pack_w(w_hh_b),
        "gbias": np.ascontiguousarray(gb),
        "brelu": np.ascontiguousarray(
            np.asarray(bias, np.float32).reshape(NK, HC).T),
    }


def _run(in_maps, trace=False, **kw):
    nc = build_program()
    return run_bass_kernel_spmd(nc, in_maps, list(range(NCORES)), trace=trace, **kw)


def kernel(x, batch, num_moles, max_len, bias, w_ih_f, w_hh_f, b_ih_f, b_hh_f,
           w_ih_b, w_hh_b, b_ih_b, b_hh_b):
    x = np.asarray(x, np.float32)
    batch = np.asarray(batch)
    assert int(num_moles) == B and int(max_len) == L
    assert x.shape == (B * L, H)
    expected_batch = np.repeat(np.arange(B, dtype=batch.dtype), L)
    assert np.array_equal(batch, expected_batch), \
        "kernel assumes uniform 48-length molecules"

    shared = _prep_shared_inputs(
        bias, w_ih_f, w_hh_f, b_ih_f, b_hh_f,
        w_ih_b, w_hh_b, b_ih_b, b_hh_b)

    in_maps = [dict(shared, x=np.ascontiguousarray(x[c * NLOC:(c + 1) * NLOC]))
               for c in range(NCORES)]
    res = _run(in_maps).results
    return np.concatenate([res[c]["y"] for c in range(NCORES)], axis=0)
